# revision 14
# baseline (speedup 1.0000x reference)
"""Trainium2 Bass kernel for blocked-modality BertSelfAttention (fp8 edition).

Problem: inputs [4, 2048, 768]; per-head QKV weights [12, 256, 64] where head h
uses modality block m = h // 4 of the hidden dim (768 = 3 x 256); standard
softmax attention per head; output [4, 2048, 768] (heads concatenated).

Sharding: 8 cores = 4 batches x 2 head-groups (heads 0-5 / 6-11); each core
does 6 heads = 3 pairs sharing modality slices.

Per-core pipeline (shaped around fp8 DoubleRow matmuls, which run the PE at
2x and fold two 128-deep k-tiles per call):
  - Q/K projections: fp8 DoubleRow (x fp8 [128,2dc,S] x W fp8 [128,2dc,64])
    into [64, 512] psum tiles laid out as the [32-partition, 2-dim-half] fold
    the DoubleRow scores need. Q's bias is preloaded into PSUM by a rank-1
    matmul (K needs no bias: its query-constant score term cancels in
    softmax); evacs are plain dtype-cast copies (q~ = fp8(8(Q+bq)),
    k~ = fp8(8K)), so the scores psum = 512*s'.
  - scores: per (t, head) one fp8 DoubleRow matmul [32,2,128]x[32,2,512]
    -> psum [128, 512]; both heads of a t share one [128, 2, 512] psum tile.
  - exp, split per key-tile t between both elementwise engines:
      ACT tiles (10/16): p = exp(s') as bf16 -> ctx via bf16 matmuls.
      DVE tiles (6/16): P' = expm1(s') ~ deg-3 poly custom DVE op, fp8 out
      -> ctx via fp8 DoubleRow over t-pairs (quarter PE cost), plus rank-1
      corrections Sum_k V (host-exact f64) that also cancel the fp8-V
      common-mode quantization error (P' couples V noise via p-1, not p).
  - V tiles stored as 16*(V+bv) with a ones-column of 16 (fused denominator);
    out = num/den needs no separate bias add in the epilogue.
  - epilogue per 128-row block: PE transpose, DVE reciprocal of the sums row,
    DVE tensor_scalar multiply into the staging buffer, DMA out.
"""

import sys

for _p in ("/opt/trn_rl_repo",):
    if _p not in sys.path:
        sys.path.insert(0, _p)

import numpy as np
import ml_dtypes

import concourse.bass as bass
import concourse.mybir as mybir
from concourse import bacc, masks
from concourse.bass_utils import run_bass_kernel_spmd
from concourse.tile import TileContext

F32 = mybir.dt.float32
F32R = mybir.dt.float32r
BF16 = mybir.dt.bfloat16
F8 = mybir.dt.float8e4
DR = mybir.MatmulPerfMode.DoubleRow

# ---- custom DVE op: expm1(x/512) via deg-3 minimax poly -------------------
import re as _re

from concourse.dve_ops import OPS as _DVE_OPS
from concourse.dve_ops import (
    CUSTOM_DVE_SPECS as _DVE_SPECS,
    _CUSTOM_DVE_ROW_BASE,
    _SUB_OPCODE_FOR_NAME,
    DveOp as _DveOp,
)
from concourse.dve_spec import C0, C1, C2, Spec, Src0

# minimax of expm1 on [-0.8, 0.8]; max abs err 6.8e-3 (comparable to the
# fp8-P' quantization noise; both average out over the 2048-key context sum)
_A1, _A2, _A3 = 0.99892253, 0.51663785, 0.1737657
SSC = 512.0  # scores psum = 512 * s'
PC = (_A1 / SSC, _A2 / SSC**2, _A3 / SSC**3)


def _expm1_ref(in0, in1, s0, s1, imm2):
    return (((imm2 * in0 + s1) * in0 + s0) * in0).astype(np.float32)


def _register_expm1():
    name = "EXPM1_POLY_ANT"
    spec = Spec(body=((C2 * Src0 + C1) * Src0 + C0) * Src0, reference=_expm1_ref)
    if name in _SUB_OPCODE_FOR_NAME:
        for op in _DVE_OPS:
            if op.name == name:
                return op
    probe = _DveOp(name, spec, subdim=False, uops_sha={})
    _DVE_OPS.append(probe)
    _SUB_OPCODE_FOR_NAME[name] = _CUSTOM_DVE_ROW_BASE + len(_DVE_OPS) - 1
    _DVE_SPECS[name] = spec
    from concourse.dve_table_gen import dve_ver_for
    ver = dve_ver_for("TRN2")
    try:
        probe.compile(ver)
        return probe
    except ValueError as e:
        m = _re.search(r"v\d: ([0-9a-f]+)", str(e)) or _re.search(
            r"\(([0-9a-f]{8,})", str(e))
        if m is None:
            raise
        op = _DveOp(name, spec, subdim=False, uops_sha={ver: m.group(1)})
        _DVE_OPS[-1] = op
        return op


EXPM1_OP = _register_expm1()

B, S, HID = 4, 2048, 768
H, DBLK, HD = 12, 256, 64
NCORES = 8
HPC = 6           # heads per core
NPAIR = 3         # head pairs per core
SCH = 512         # s-chunk width
NSCH = S // SCH   # 4
NT = S // 128     # 16 key tiles
NSB = S // 128    # 16 output s-blocks

# key-tile -> engine assignment. DVE tiles pair up for DoubleRow ctx
# (a trailing singleton runs as a plain fp8 matmul).
D_PAIRS = ((1, 3), (5, 7), (9, 11), (13,))
D_SET = {t: (j, u) for j, pr in enumerate(D_PAIRS) for u, t in enumerate(pr)}
A_LIST = [t for t in range(NT) if t not in D_SET]
A_POS = {t: i for i, t in enumerate(A_LIST)}
ND = len(D_PAIRS)         # DVE t-pairs
NA = len(A_LIST)          # ACT tiles
N_DVE_KEYS = 2 * ND * 128

QS = 8.0    # stored q~/k~ = 8*(..)
VS = 16.0   # stored v~ = 16*(V+bv); ones column = 16


def build_bass():
    nc = bacc.Bacc(None, target_bir_lowering=False)

    xt8 = nc.dram_tensor("xt8", [NPAIR, 2, 128, S], F8, kind="ExternalInput")
    xtb = nc.dram_tensor("xtb", [2, 2, 128, S], BF16, kind="ExternalInput")
    wq8 = nc.dram_tensor("wq8", [NPAIR, 2, 128, 2, 64], F8, kind="ExternalInput")
    wk8 = nc.dram_tensor("wk8", [NPAIR, 2, 128, 2, 64], F8, kind="ExternalInput")
    wv0 = nc.dram_tensor("wv0", [2, 128, 256], BF16, kind="ExternalInput")
    wv1 = nc.dram_tensor("wv1", [2, 128, 128], BF16, kind="ExternalInput")
    bqf = nc.dram_tensor("bqf", [1, NPAIR, 2, 64], F32R, kind="ExternalInput")
    corr = nc.dram_tensor("corr", [1, NPAIR, 2, 128], F32R, kind="ExternalInput")
    ones = nc.dram_tensor("ones", [1, SCH], F32R, kind="ExternalInput")
    bv0 = nc.dram_tensor("bv0", [256], F32, kind="ExternalInput")
    bv1 = nc.dram_tensor("bv1", [128], F32, kind="ExternalInput")
    out = nc.dram_tensor("out", [S, HPC * HD], F32, kind="ExternalOutput")

    with TileContext(nc) as tc:
        with (
            tc.tile_pool(name="const", bufs=1) as cpool,
            tc.tile_pool(name="xts", bufs=2) as xtpool,
            tc.tile_pool(name="cs", bufs=3) as cspool,
            tc.tile_pool(name="rcp", bufs=8) as rcpool,
            tc.tile_pool(name="ps_sc", bufs=2, space="PSUM") as pssc,
            tc.tile_pool(name="ps_ctx", bufs=1, space="PSUM") as psc,
            tc.tile_pool(name="ps_x", bufs=1, space="PSUM") as psx,
            tc.tile_pool(name="ps_fin", bufs=1, space="PSUM") as psf,
        ):
            # ---- constants / persistent arrays ----
            wq_sb = cpool.tile([128, NPAIR, 2, 2, 64], F8)
            wk_sb = cpool.tile([128, NPAIR, 2, 2, 64], F8)
            wv0_sb = cpool.tile([128, 2, 256], BF16)
            wv1_sb = cpool.tile([128, 2, 128], BF16)
            bqf_sb = cpool.tile([1, NPAIR, 2, 64], F32R)
            corr_sb = cpool.tile([1, NPAIR, 2, 128], F32R)
            ones512 = cpool.tile([1, SCH], F32R)
            bvb0 = cpool.tile([128, 256], F32)
            bvb1 = cpool.tile([128, 128], F32)
            id66 = cpool.tile([66, 66], F32R)
            id66_32 = cpool.tile([66, 66], F32)
            out_stage = cpool.tile([128, NSB, HPC * HD], F32)

            q_all = cpool.tile([64, NPAIR, 2, S], F8)
            k_all = cpool.tile([64, NPAIR, 2, S], F8)
            v8 = cpool.tile([128, ND, 2, NPAIR, 2, 128], F8)
            vb = cpool.tile([128, NA, NPAIR, 2, 66], BF16)
            p8_all = cpool.tile([128, ND, 2, 2, SCH], F8)
            pb_all = cpool.tile([128, NA, 2, SCH], BF16)
            xtb_sb = cpool.tile([128, 2, 2, S], BF16)  # [slot, dc]

            def load_xt8(p, s_lo=0, s_hi=S):
                t = xtpool.tile([128, 2, S], F8, tag="xt8", name="xt8_sb")
                nc.sync.dma_start(
                    out=t[:, :, s_lo:s_hi],
                    in_=xt8[p, :, :, s_lo:s_hi].rearrange("c q s -> q c s"))
                return t

            # ---- load order: pair-0 x first so its proj starts early ----
            xt_cur = load_xt8(0, 0, SCH)
            nc.sync.dma_start(out=bqf_sb[:, :, :, :], in_=bqf[:, :, :, :])
            nc.sync.dma_start(out=ones512[:, :], in_=ones[:, :])
            nc.sync.dma_start(out=corr_sb[:, :, :, :], in_=corr[:, :, :, :])
            nc.sync.dma_start(out=wq_sb[:, :, :, :, :],
                              in_=wq8.rearrange("p c q h m -> q p c h m"))
            nc.sync.dma_start(out=wk_sb[:, :, :, :, :],
                              in_=wk8.rearrange("p c q h m -> q p c h m"))
            nc.sync.dma_start(
                out=xt_cur[:, :, SCH:],
                in_=xt8[0, :, :, SCH:].rearrange("c q s -> q c s"))
            nc.sync.dma_start(out=xtb_sb[:, 0, :, :],
                              in_=xtb[0, :, :, :].rearrange("c q s -> q c s"))
            nc.sync.dma_start(out=wv0_sb[:, :, :],
                              in_=wv0.rearrange("c q m -> q c m"))
            nc.sync.dma_start(out=wv1_sb[:, :, :],
                              in_=wv1.rearrange("c q m -> q c m"))
            nc.sync.dma_start(out=xtb_sb[:, 1, :, :],
                              in_=xtb[1, :, :, :].rearrange("c q s -> q c s"))
            nc.sync.dma_start(out=bvb0[:, :],
                              in_=bass.AP(bv0, 0, [[0, 128], [1, 256]]))
            nc.sync.dma_start(out=bvb1[:, :],
                              in_=bass.AP(bv1, 0, [[0, 128], [1, 128]]))

            masks.make_identity(nc, id66_32[:, :])
            nc.vector.tensor_copy(id66[:, :], id66_32[:, :])
            # ones columns (=VS) and zero pad in the V tiles; fp8/bf16
            # strided memset fails codegen, so stage in f32 and copy-cast
            oz = cpool.tile([128, NA * NPAIR * 2, 64], F32)
            nc.vector.memset(oz[:, :, :], 0.0)
            nc.vector.memset(oz[:, :, 0:1], VS)
            v8f = v8[:, :, :, :, :, :].rearrange("q j u p h e -> q (j u p h) e")
            vbf = vb[:, :, :, :, :].rearrange("q i p h e -> q (i p h) e")
            nc.vector.tensor_copy(v8f[:, :, 64:128],
                                  oz[:, 0:ND * 2 * NPAIR * 2, :])
            nc.vector.tensor_copy(vbf[:, :, 64:66], oz[:, :, 0:2])

            # ---- projection pieces for (pair, sc): list of closures --------
            def proj_pieces(p, xt_sb, sc):
                s0, s1 = sc * SCH, (sc + 1) * SCH

                def q_piece(i):
                    tq = psx.tile([64, SCH], F32, tag="x", name="tq")
                    nc.tensor.matmul(
                        tq[:, :], bqf_sb[0:1, p, i, :], ones512[0:1, :],
                        start=True, stop=False, skip_group_check=True)
                    nc.tensor.matmul(
                        tq[:, :], wq_sb[:, p, :, i, :], xt_sb[:, :, s0:s1],
                        start=False, stop=True, perf_mode=DR,
                        skip_group_check=True)
                    nc.scalar.copy(q_all[:, p, i, s0:s1], tq[:, :])

                def k_piece(i):
                    tk = psx.tile([64, SCH], F32, tag="x", name="tk")
                    nc.tensor.matmul(
                        tk[:, :], wk_sb[:, p, :, i, :], xt_sb[:, :, s0:s1],
                        start=True, stop=True, perf_mode=DR)
                    nc.vector.tensor_copy(k_all[:, p, i, s0:s1], tk[:, :])

                pieces = [lambda: q_piece(0), lambda: q_piece(1),
                          lambda: k_piece(0), lambda: k_piece(1)]
                # V pieces: pair 0 emits slot-0 (pairs 0+1, 4 heads); pair 2
                # emits slot-1 (pair 2, 2 heads). 4 t-tiles per sc quarter.
                if p == 1:
                    return pieces
                slot = 0 if p == 0 else 1
                wv_sb = wv0_sb if slot == 0 else wv1_sb
                bvb = bvb0 if slot == 0 else bvb1
                npr = 2 - slot  # pairs covered
                ncol = 128 * npr * 2 // 2  # 256 / 128
                for t in range(4 * sc, 4 * sc + 4):
                    def v_piece(t=t):
                        tv = psx.tile([128, ncol], F32, tag="x", name="tv")
                        for dc in range(2):
                            nc.tensor.matmul(
                                tv[:, :],
                                xtb_sb[:, slot, dc, t * 128:(t + 1) * 128],
                                wv_sb[:, dc, :], start=(dc == 0), stop=(dc == 1))
                        if t in D_SET:
                            j, u = D_SET[t]
                            dst = v8[:, j, u, 2 * slot:2 * slot + npr, :, 0:64]
                        else:
                            i = A_POS[t]
                            dst = vb[:, i, 2 * slot:2 * slot + npr, :, 0:64]
                        nc.vector.scalar_tensor_tensor(
                            dst.rearrange("q p h e -> q (p h) e"),
                            tv[:, :].rearrange("q (f e) -> q f e", f=2 * npr),
                            VS,
                            bvb[:, 0:ncol].rearrange("q (f e) -> q f e",
                                                     f=2 * npr),
                            mybir.AluOpType.mult, mybir.AluOpType.add)
                    pieces.append(v_piece)
                return pieces

            # ---- epilogue pieces for (pair, sc): each piece transposes two
            # blocks, then the NEXT piece runs their rcp/stt (so the DVE ops
            # arrive with their PE producer already executed -- an rcp queued
            # behind an unexecuted transpose stalls the in-order DVE queue)
            def epi_pieces(p, sc, ctx_sb):
                fins = {}

                def tr_piece(g):
                    # two transposes share one psum bank as ONE accumulation
                    # group (start zeroes the whole 2KB zero-region, so each
                    # bank holds exactly one group)
                    fin2 = psf.tile([128, 2, 66], F32R, tag="f", name="fin2")
                    fins[g] = fin2
                    for w, bb in enumerate((2 * g, 2 * g + 1)):
                        hh, blk = bb // 4, bb % 4
                        nc.tensor.matmul(
                            fin2[:, w, :],
                            ctx_sb[:, hh, blk * 128:(blk + 1) * 128],
                            id66[:, :], is_transpose=True,
                            start=(w == 0), stop=(w == 1),
                            skip_group_check=True)

                def div_piece(g):
                    fin2 = fins.pop(g)
                    for w, bb in enumerate((2 * g, 2 * g + 1)):
                        hh, blk = bb // 4, bb % 4
                        hl = 2 * p + hh
                        sb = sc * (SCH // 128) + blk
                        rc = rcpool.tile([128, 1], F32, tag="rc")
                        nc.vector.reciprocal(rc[:, :], fin2[:, w, 64:65])
                        nc.vector.tensor_scalar_mul(
                            out_stage[:, sb, hl * 64:(hl + 1) * 64],
                            fin2[:, w, 0:64], rc[:, :])

                def dma_piece():
                    nc.sync.dma_start(
                        out=out.rearrange("(t q) c -> q t c", q=128)[
                            :, sc * 4:(sc + 1) * 4, p * 2 * HD:(p + 1) * 2 * HD],
                        in_=out_stage[:, sc * 4:(sc + 1) * 4,
                                      p * 2 * HD:(p + 1) * 2 * HD])

                def mk(fs):
                    def run():
                        for f in fs:
                            f()
                    return run
                pieces = [mk([lambda: tr_piece(0)]),
                          mk([lambda: div_piece(0), lambda: tr_piece(1)]),
                          mk([lambda: div_piece(1), lambda: tr_piece(2)]),
                          mk([lambda: div_piece(2), lambda: tr_piece(3)]),
                          mk([lambda: div_piece(3), dma_piece])]
                return pieces

            # ---- the t-loop for (pair, sc) ----
            def emit_tloop(p, sc, pieces):
                s0 = sc * SCH
                ctx_ps = psc.tile([128, 2, SCH], F32, tag="c", name="ctx_ps")
                # rank-1 corrections open (and zero) each half's accumulation
                for hh in range(2):
                    nc.tensor.matmul(
                        ctx_ps[:, hh, :], corr_sb[0:1, p, hh, :],
                        ones512[0:1, :],
                        start=True, stop=False, skip_group_check=True)

                n_groups = NA + ND
                n_done = 0
                pend = []

                def emit_ctx(kind, idx):
                    nonlocal n_done
                    n_done += 1
                    last = n_done == n_groups
                    for hh in range(2):
                        if kind == "a":
                            nc.tensor.matmul(
                                ctx_ps[0:66, hh, :], vb[:, idx, p, hh, :],
                                pb_all[:, idx, hh, :],
                                start=False, stop=last, skip_group_check=True)
                        elif len(D_PAIRS[idx]) == 2:
                            nc.tensor.matmul(
                                ctx_ps[:, hh, :],
                                v8[:, idx, :, p, hh, :],
                                p8_all[:, idx, :, hh, :],
                                start=False, stop=last, perf_mode=DR,
                                skip_group_check=True)
                        else:
                            nc.tensor.matmul(
                                ctx_ps[:, hh, :],
                                v8[:, idx, 0, p, hh, :],
                                p8_all[:, idx, 0, hh, :],
                                start=False, stop=last,
                                skip_group_check=True)

                pi = 0
                for t in range(NT):
                    sc_ps = pssc.tile([128, 2, SCH], F32, tag="sc", name="sc_ps")
                    for hh in range(2):
                        nc.tensor.matmul(
                            sc_ps[:, hh, :],
                            k_all[hh * 32:(hh + 1) * 32, p, :,
                                  t * 128:(t + 1) * 128],
                            q_all[hh * 32:(hh + 1) * 32, p, :, s0:s0 + SCH],
                            start=True, stop=True, perf_mode=DR)
                    if t in D_SET:
                        j, u = D_SET[t]
                        nc.vector._custom_dve(
                            EXPM1_OP, out=p8_all[:, j, u, :, :],
                            in0=sc_ps[:, :, :],
                            s0=PC[0], s1=PC[1], imm2=PC[2])
                        if u == len(D_PAIRS[j]) - 1:
                            pend.append(("d", j))
                    else:
                        i = A_POS[t]
                        nc.scalar.activation(
                            pb_all[:, i, :, :], sc_ps[:, :, :],
                            mybir.ActivationFunctionType.Exp, scale=1.0 / SSC)
                        pend.append(("a", i))
                    # interleaved pieces, BEFORE ctx pops (ctx may
                    # consume V written by a just-emitted piece)
                    budget = -(-(len(pieces)) // 12)  # drain in ~12 of 16 t's
                    for _ in range(budget):
                        if pi < len(pieces):
                            pieces[pi]()
                            pi += 1
                    # defer ctx by 2 groups so freshly-freed psum refills first
                    while len(pend) > 2:
                        emit_ctx(*pend.pop(0))
                while pi < len(pieces):
                    pieces[pi]()
                    pi += 1
                for args in pend:
                    emit_ctx(*args)
                ctx_sb = cspool.tile([66, 2, SCH], F32R, tag="cs", name="ctx_sb")
                nc.scalar.copy(ctx_sb[:, :, :], ctx_ps[0:66, :, :])
                return ctx_sb

            # ---- main loop ----
            # pair-0 proj: sc 0 upfront, sc 1-3 interleaved into loop (0, 0)
            # (its ctx needs V for all 16 key tiles).
            for piece in proj_pieces(0, xt_cur, 0):
                piece()
            pending = None
            xt_nxt = None
            for p in range(NPAIR):
                if p + 1 < NPAIR:
                    xt_nxt = load_xt8(p + 1)
                for sc in range(NSCH):
                    extra = []
                    if p == 0 and sc == 0:
                        for s2 in range(1, NSCH):
                            extra += proj_pieces(0, xt_cur, s2)
                    if pending is not None:
                        extra += epi_pieces(*pending)
                    if p + 1 < NPAIR:
                        extra += proj_pieces(p + 1, xt_nxt, sc)
                    ctx_sb = emit_tloop(p, sc, extra)
                    pending = (p, sc, ctx_sb)
            for piece in epi_pieces(*pending):
                piece()

    nc.finalize()
    return nc


_NC = None


def _get_nc():
    global _NC
    if _NC is None:
        _NC = build_bass()
    return _NC


def _f8(a):
    return np.asarray(a, ml_dtypes.float8_e4m3fn)


def _bf(a):
    return np.asarray(a, ml_dtypes.bfloat16)


def _prep_core_inputs(inputs, Wq, bq, Wk, bk, Wv, bv, b, g):
    heads = list(range(g * HPC, (g + 1) * HPC))
    pairs = [(heads[0], heads[1]), (heads[2], heads[3]), (heads[4], heads[5])]
    if pairs[0][0] // 4 != pairs[1][0] // 4:
        pairs = [pairs[1], pairs[2], pairs[0]]
    assert pairs[0][0] // 4 == pairs[1][0] // 4
    head_order = [h for pr in pairs for h in pr]
    xT = np.ascontiguousarray(inputs[b].T)  # [HID, S]

    xt8 = np.empty((NPAIR, 2, 128, S), ml_dtypes.float8_e4m3fn)
    xtb = np.empty((2, 2, 128, S), ml_dtypes.bfloat16)
    wq8 = np.empty((NPAIR, 2, 128, 2, 64), ml_dtypes.float8_e4m3fn)
    wk8 = np.empty((NPAIR, 2, 128, 2, 64), ml_dtypes.float8_e4m3fn)
    bqf = np.empty((1, NPAIR, 2, 64), np.float32)
    corr = np.empty((1, NPAIR, 2, 128), np.float32)

    dve_keys = np.zeros(S, bool)
    for pr in D_PAIRS:
        for t in pr:
            dve_keys[t * 128:(t + 1) * 128] = True

    for p in range(NPAIR):
        hA, hB = pairs[p]
        mod = hA // 4
        for dc in range(2):
            d0 = mod * DBLK + dc * 128
            xt8[p, dc] = _f8(xT[d0:d0 + 128])
            for i in range(2):
                # weight cols: [hA dims 32i..32i+32 | hB dims 32i..32i+32]
                wq8[p, dc, :, i, 0:32] = _f8(
                    QS * Wq[hA][dc * 128:(dc + 1) * 128, 32 * i:32 * i + 32])
                wq8[p, dc, :, i, 32:64] = _f8(
                    QS * Wq[hB][dc * 128:(dc + 1) * 128, 32 * i:32 * i + 32])
                wk8[p, dc, :, i, 0:32] = _f8(
                    QS * Wk[hA][dc * 128:(dc + 1) * 128, 32 * i:32 * i + 32])
                wk8[p, dc, :, i, 32:64] = _f8(
                    QS * Wk[hB][dc * 128:(dc + 1) * 128, 32 * i:32 * i + 32])
        for i in range(2):
            bqf[0, p, i, 0:32] = QS * bq[hA][32 * i:32 * i + 32]
            bqf[0, p, i, 32:64] = QS * bq[hB][32 * i:32 * i + 32]
        # host-exact rank-1 ctx correction over DVE-assigned keys
        x64 = inputs[b][:, mod * DBLK:(mod + 1) * DBLK].astype(np.float64)
        for hh, h in enumerate((hA, hB)):
            Vt = x64 @ Wv[h].astype(np.float64) + bv[h].astype(np.float64)
            corr[0, p, hh, :] = 0.0
            corr[0, p, hh, 0:64] = VS * Vt[dve_keys].sum(0)
            corr[0, p, hh, 64] = VS * N_DVE_KEYS

    mods = (pairs[0][0] // 4, pairs[2][0] // 4)
    for slot, mod in enumerate(mods):
        for dc in range(2):
            d0 = mod * DBLK + dc * 128
            xtb[slot, dc] = _bf(xT[d0:d0 + 128])

    h4 = [pairs[0][0], pairs[0][1], pairs[1][0], pairs[1][1]]
    wv0 = np.empty((2, 128, 256), ml_dtypes.bfloat16)
    wv1 = np.empty((2, 128, 128), ml_dtypes.bfloat16)
    for dc in range(2):
        wv0[dc] = _bf(np.concatenate(
            [Wv[h][dc * 128:(dc + 1) * 128] for h in h4], axis=1))
        wv1[dc] = _bf(np.concatenate(
            [Wv[h][dc * 128:(dc + 1) * 128] for h in pairs[2]], axis=1))
    bv0 = (VS * np.concatenate([bv[h] for h in h4])).astype(np.float32)
    bv1 = (VS * np.concatenate([bv[h] for h in pairs[2]])).astype(np.float32)

    return ({"xt8": xt8, "xtb": xtb, "wq8": wq8, "wk8": wk8,
             "wv0": wv0, "wv1": wv1, "bqf": bqf, "corr": corr,
             "ones": np.ones((1, SCH), np.float32),
             "bv0": bv0, "bv1": bv1}, head_order)


def run_cores(inputs, Wq, bq, Wk, bk, Wv, bv, **kwargs):
    args = [np.asarray(a, np.float32) for a in (inputs, Wq, bq, Wk, bk, Wv, bv)]
    inputs, Wq, bq, Wk, bk, Wv, bv = args
    in_maps = []
    orders = []
    for core in range(NCORES):
        b, g = core // 2, core % 2
        m, order = _prep_core_inputs(inputs, Wq, bq, Wk, bk, Wv, bv, b, g)
        in_maps.append(m)
        orders.append(order)
    nc = _get_nc()
    res = run_bass_kernel_spmd(nc, in_maps, core_ids=list(range(NCORES)), **kwargs)
    full = np.empty((B, S, H * HD), np.float32)
    for core in range(NCORES):
        b = core // 2
        o = res.results[core]["out"]
        for hl, h in enumerate(orders[core]):
            full[b, :, h * HD:(h + 1) * HD] = o[:, hl * HD:(hl + 1) * HD]
    return full, res


def kernel(inputs, Wq, bq, Wk, bk, Wv, bv):
    full, _ = run_cores(inputs, Wq, bq, Wk, bk, Wv, bv)
    return full
